# revision 15
# baseline (speedup 1.0000x reference)
"""Trainium2 Bass kernel for gaussian-weighted box-feature scatter (pooling).

Math (from the reference):
    out[c,h,w] = (1/N) * sum_n box_feats[c,n] * gmaps[n,h,w]
with gmaps separable:
    gmaps[n,h,w] = gy[n,h] * gx[n,w]

Host (tiny, O(N*C + N*(H+W))): box corner math, one bilinear sample per box
(box_feats [C,N]), and the two 1-D gaussian profiles gy [N,H], gx [N,W].

Device (heavy, O(C*H*W)): rank-N reconstruction
    out[c,h,w] = sum_n B_h[n,c] * gx[n,w],   B_h[n,c] = (A_T[n,c]/N)*gy[n,h]
as per-h fp16 matmuls on the PE (stationary B_h halves, moving gx; the
per-matmul LDWEIGHTS is hidden by the PE's background weight buffer), PSUM
f32 accumulate, then fp16 PSUM->SBUF copy-casts and fp16 DMA writes to HBM.
The f32->fp16 output cast is the big win: the kernel is write-bandwidth
bound (per-core HBM ~358 GB/s), so halving output bytes halves the floor
(16.8 MB/core ~= 47 us). Host upcasts fp16 -> f32 when gathering.

Engine split (all under the ~47 us DMA window):
  GpSimd: 64x tensor_scalar B_h [20,256] fp16        (~0.3-0.5 us each)
  PE:     128x matmul [20,128]x[20,512] fp16         (~0.22 us warm)
  DVE:    even-h PSUM->SBUF double copies [128,1024] (~1.1 us each, 32x)
  Scalar: odd-h double copies                        (~1.2 us each, 32x)
  SP:     1 input DMA + 7 chunked output DMAs (exactly 8 HWDGE lanes)

Each psum tile spans 2 banks: mm half0 -> cols 0:512, half1 -> 512:1024,
so one copy per h moves both c-halves. Output chunks [2,2,4,8,16,16,16]
h-rows: the first DMA issues ~2.5 us after the input lands, and every
chunk DMA covers both halves via a strided dram AP.

Sharding: H split across the 8 cores (64 rows each) - fully local.
"""

import numpy as np
from contextlib import ExitStack

from concourse import bass, tile, mybir
from concourse.tile import add_dep_helper
from concourse.bass_utils import run_bass_kernel_spmd

# Problem shapes (hardcoded per the task contract).
C, H, W = 256, 512, 512
N = 20
N_CORES = 8
HS = H // N_CORES          # 64 rows of the output per core
CHUNKS = [2, 2, 4, 8, 16, 16, 16]   # h-rows per output DMA chunk
F32 = mybir.dt.float32
F16 = mybir.dt.float16

VOXEL = (0.4, 0.4, 4.0)
LIDAR_RANGE = (-102.4, -102.4, -3.0, 102.4, 102.4, 1.0)
DOWNSAMPLE = 1

_PROG = None          # cached Bass program
LAST_RESULTS = None   # BassKernelResults of the most recent run (for test.py)


def _host_factors(pred_box_infra, infra_features):
    """Per-box scalars, bilinear-sampled box features and separable gaussian
    profiles - all tiny. Coordinate math in float32 to match the reference
    bit-for-bit where it matters (floor/clip decisions)."""
    boxes = pred_box_infra[:N].astype(np.float32)
    feat = infra_features[0]                      # [C,H,W] float32
    l_corner = boxes.min(axis=1)                  # [N,3]
    r_corner = boxes.max(axis=1)
    sx = np.float32(VOXEL[0] * DOWNSAMPLE)
    sy = np.float32(VOXEL[1] * DOWNSAMPLE)
    x1 = (l_corner[:, 0] - np.float32(LIDAR_RANGE[0])) / sx
    y1 = (l_corner[:, 1] - np.float32(LIDAR_RANGE[1])) / sy
    x2 = (r_corner[:, 0] - np.float32(LIDAR_RANGE[0])) / sx
    y2 = (r_corner[:, 1] - np.float32(LIDAR_RANGE[1])) / sy
    bev_size = (y2 - y1) * (x2 - x1)              # [N]
    cx = np.float32(0.5) * (x1 + x2)
    cy = np.float32(0.5) * (y1 + y2)

    # bilinear sample at (cy, cx), matching the reference's clip/floor
    y = np.clip(cy, 0.0, H - 1.0).astype(np.float32)
    x = np.clip(cx, 0.0, W - 1.0).astype(np.float32)
    yl = np.floor(y).astype(np.int32)
    xl = np.floor(x).astype(np.int32)
    yh = np.minimum(yl + 1, H - 1)
    xh = np.minimum(xl + 1, W - 1)
    ly = (y - yl).astype(np.float64)[None, :]     # [1,N]
    lx = (x - xl).astype(np.float64)[None, :]
    g = lambda yi, xi: feat[:, yi, xi].astype(np.float64)   # [C,N]
    box_feats = (g(yl, xl) * (1 - ly) * (1 - lx)
                 + g(yl, xh) * (1 - ly) * lx
                 + g(yh, xl) * ly * (1 - lx)
                 + g(yh, xh) * ly * lx)           # [C,N] float64

    denom = 2.0 * bev_size.astype(np.float64) ** 2          # [N]
    hh = np.arange(H, dtype=np.float64)
    ww = np.arange(W, dtype=np.float64)
    gy = np.exp(-((hh[None, :] - x1.astype(np.float64)[:, None]) ** 2) / denom[:, None])
    gx = np.exp(-(ww[None, :] ** 2) / denom[:, None])

    a_t = np.ascontiguousarray((box_feats / N).T.astype(np.float32))  # [N,C]
    return a_t, gy.astype(np.float32), gx.astype(np.float32)


def _build_program():
    nc = bass.Bass("TRN2", target_bir_lowering=False, debug=False,
                   num_devices=N_CORES)
    # params = concat([a_t [N,C], gy [N,HS], gx [N,W]], axis=1): one DMA,
    # one semaphore.
    PF = C + HS + W
    params = nc.dram_tensor("params", [N, PF], F32, kind="ExternalInput").ap()
    out = nc.dram_tensor("out", [C, HS, W], F16, kind="ExternalOutput").ap()
    # [c, b, h, w] view with c the 128-partition dim and b the c-half.
    out_v = out.rearrange("(b c) h w -> c b h w", b=2)

    with ExitStack() as ctx:
        tc = ctx.enter_context(tile.TileContext(nc))
        const = ctx.enter_context(tc.tile_pool(name="const", bufs=1))
        # B_h tiles: one slot per h (no recycling — a recycled slot would put
        # a second sync wait on the TensorScalar, whose ISA struct holds one).
        bpool = ctx.enter_context(tc.tile_pool(name="bh", bufs=HS))
        ppool = ctx.enter_context(tc.tile_pool(name="psum", bufs=4, space="PSUM"))
        # One stage pool per chunk size; bufs == #chunks of that size, so
        # stage slots are never recycled (no release waits needed at all).
        spools = {}
        for s in sorted(set(CHUNKS)):
            spools[s] = ctx.enter_context(
                tc.tile_pool(name=f"stage{s}", bufs=CHUNKS.count(s)))

        p_sb = const.tile([N, PF], F32)
        in_dma = nc.sync.dma_start(p_sb[:], params[:])
        a_sb = p_sb[:, 0:C]
        gy_sb = p_sb[:, C:C + HS]
        gx_sb = p_sb[:, C + HS:PF]
        # fp16 copy of gx for the moving matmul operand.
        gx16 = const.tile([N, W], F16)
        nc.vector.tensor_copy(gx16[:], gx_sb)
        # One scratch column per chunk for the DVE joiner memsets.
        scratch = const.tile([128, len(CHUNKS)], F32)

        tail_deps = [in_dma.ins]
        h = 0
        for ci, s in enumerate(CHUNKS):
            # Stage layout per partition: [b(half)][h][w] so the DMA's SBUF
            # side merges (h,w) into one contiguous run and balances at 3D.
            stage = spools[s].tile([128, 2 * s * W], F16, tag="stage")
            stage_v = stage[:].rearrange("p (b h w) -> p b h w", b=2, h=s)
            for l in range(s):
                b = bpool.tile([N, C], F16)
                bts = nc.gpsimd.tensor_scalar_mul(b[:], a_sb, gy_sb[:, h:h + 1])
                ps = ppool.tile([128, 2 * W], F32, tag="ps")
                nc.tensor.matmul(ps[:, 0:W], b[:, 0:128], gx16[:],
                                 start=True, stop=True)
                mm = nc.tensor.matmul(ps[:, W:2 * W], b[:, 128:256], gx16[:],
                                      start=True, stop=True)
                ps_v = ps[:].rearrange("p (b w) -> p b w", b=2)
                if h % 2 == 0:
                    cp = nc.vector.tensor_copy(stage_v[:, :, l, :], ps_v)
                else:
                    cp = nc.scalar.copy(stage_v[:, :, l, :], ps_v)
                    last_act_cp = cp
                h += 1
            # The chunk DMA depends on copies from both engines, but the
            # DMA descriptor holds ONE sync wait. Emit a DVE joiner that
            # waits on the chunk's last Act copy (DVE program order already
            # covers the DVE copies); the DMA then waits only the joiner's
            # DVE tick, and the implied Act wait is elided post-assignment.
            joiner = nc.vector.memset(scratch[:, ci:ci + 1], 0.0)
            add_dep_helper(joiner.ins, last_act_cp.ins, sync=True,
                           reason="chunk copy joiner")
            dma = nc.sync.dma_start(out_v[:, :, h - s:h, :], stage_v)
            add_dep_helper(dma.ins, joiner.ins, sync=True,
                           reason="dma waits joiner")
            tail_deps.append(dma.ins)

        # Tail drain pre-cover: one single-wait SP nop per outstanding sem
        # so the drain itself needs no multi-wait instruction.
        tail_deps += [mm.ins, cp.ins, bts.ins, joiner.ins]
        for dep in tail_deps:
            tnop = nc.sync.nop(nofuse=True)
            add_dep_helper(tnop.ins, dep, sync=True,
                           reason="tail drain pre-cover")
    _elide_implied_waits(nc, tc)
    return nc


def _elide_implied_waits(nc, tc):
    """Several ISA structs (Matmult, TensorScalar, DMA_DIRECT2D) hold ONE
    sync wait, but Tile sometimes assigns two:
      - PSUM slot recycling puts both the PSUM->SBUF copy's tick and a WAW
        "previous writer retired" PE self-wait on the reusing matmul, yet
        the copy itself already waits for that PE tick;
      - a chunk DMA waits on both copy engines, yet its DVE joiner already
        waits the Act tick.
    A wait is provably redundant when another wait's producing instruction
    itself waits for the same semaphore at an equal-or-higher value
    (completion of the producer implies the dropped condition). Verify that
    per instruction and drop only implied waits."""
    # (sem_name, cumulative_value) -> instruction whose update reaches it.
    producer = {}
    cum = {}
    for insts in tc.ordered_instructions_by_block.values():
        for inst in insts:
            si = inst.sync_info
            if si is None:
                continue
            for u in si.on_update:
                cum[u.ant_name] = cum.get(u.ant_name, 0) + (u.update_value or 1)
                producer[(u.ant_name, cum[u.ant_name])] = inst

    def implied(keep, w):
        """True if wait `w` is implied by completion of `keep`'s producer."""
        prod = producer.get((keep.ant_name, keep.wait_value))
        if prod is None or prod.sync_info is None:
            return False
        return any(pw.ant_name == w.ant_name and pw.wait_value >= w.wait_value
                   for pw in prod.sync_info.on_wait)

    for inst in nc.inst_map.values():
        si = inst.sync_info
        if si is None or len(si.on_wait) < 2:
            continue
        waits = list(si.on_wait)
        changed = True
        while changed and len(waits) > 1:
            changed = False
            for w in waits:
                if any(k is not w and implied(k, w) for k in waits):
                    waits.remove(w)
                    changed = True
                    break
        if len(waits) != len(si.on_wait):
            si.on_wait = waits
            inst.sync_info = si


def _program():
    global _PROG
    if _PROG is None:
        _PROG = _build_program()
    return _PROG


def make_in_maps(pred_box_infra, infra_features):
    a_t, gy_full, gx = _host_factors(
        np.asarray(pred_box_infra, dtype=np.float32),
        np.asarray(infra_features, dtype=np.float32),
    )
    return [
        {
            "params": np.ascontiguousarray(np.concatenate(
                [a_t, gy_full[:, c * HS:(c + 1) * HS], gx], axis=1)),
        }
        for c in range(N_CORES)
    ]


def kernel(pred_box_infra, infra_features):
    global LAST_RESULTS
    in_maps = make_in_maps(pred_box_infra, infra_features)
    nc = _program()
    res = run_bass_kernel_spmd(nc, in_maps, core_ids=list(range(N_CORES)))
    LAST_RESULTS = res
    full = np.empty((1, C, H, W), dtype=np.float32)
    for c in range(N_CORES):
        full[0, :, c * HS:(c + 1) * HS, :] = res.results[c]["out"].astype(np.float32)
    return full


# revision 16
# speedup vs baseline: 3.3937x; 3.3937x over previous
"""Trainium2 Bass kernel for gaussian-weighted box-feature scatter (pooling).

Math (from the reference):
    out[c,h,w] = (1/N) * sum_n box_feats[c,n] * gmaps[n,h,w]
with gmaps separable:
    gmaps[n,h,w] = gy[n,h] * gx[n,w]

Host (tiny, O(N*C + N*(H+W) + N*H*C rank-factor prep)): box corner math, one
bilinear sample per box (box_feats [C,N]), the two 1-D gaussian profiles
gy [N,H], gx [N,W], and the premultiplied stationary factors
    B[n,h,c] = (box_feats[c,n]/N) * gy[n,h]   (fp16)
which ship to the device alongside gx (fp16) in one input DMA (~676 KB).

Device (heavy, O(C*H*W)): rank-N reconstruction
    out[c,h,:] = B[:,h,chalf].T @ gx
as 128 fp16 matmuls (stationary B slice via a ~105 ns standalone LDWEIGHTS
that the PE hides behind the previous matmul's streaming; moving gx), PSUM
f32 accumulate, fp16 PSUM->SBUF copy-casts, fp16 DMA writes. The f32->fp16
output is the big win: the kernel is write-bandwidth bound (per-core HBM
~358 GB/s), so halving output bytes halves the floor (16.8 MB/core ~ 47 us).
Host upcasts fp16 -> f32 while gathering.

Engine split (all under the ~47 us DMA window):
  PE:     128x (LDWEIGHTS + matmul [20,128]x[20,512] fp16)
  DVE:    even-h PSUM->SBUF double copies [128,1024] f32->fp16 (32x ~1.2 us)
  Scalar: odd-h double copies (32x ~1.1 us)
  SP:     1 input DMA + 7 chunked output DMAs (exactly 8 HWDGE lanes)

Each psum tile spans 2 banks: mm half0 -> cols 0:512, half1 -> 512:1024, so
one copy per h moves both c-halves. Output chunks [2,2,4,8,16,16,16] h-rows:
the first DMA issues ~2.5 us after the input lands; every chunk DMA covers
both halves via a strided dram AP. Per-chunk DVE "joiner" memsets plus a
post-assignment implied-wait elision keep every single-wait ISA struct
(Matmult, DMA descriptor) at one sync wait.

Sharding: H split across the 8 cores (64 rows each) - fully local.
"""

import numpy as np
from contextlib import ExitStack

from concourse import bass, tile, mybir
from concourse.tile import add_dep_helper
from concourse.bass_utils import run_bass_kernel_spmd

# Problem shapes (hardcoded per the task contract).
C, H, W = 256, 512, 512
N = 20
N_CORES = 8
HS = H // N_CORES          # 64 rows of the output per core
CHUNKS = [2, 2, 4, 8, 16, 16, 16]   # h-rows per output DMA chunk
F32 = mybir.dt.float32
F16 = mybir.dt.float16
PF = HS * C + W            # fp16 params: B[n, h, c] flattened + gx[n, w]

VOXEL = (0.4, 0.4, 4.0)
LIDAR_RANGE = (-102.4, -102.4, -3.0, 102.4, 102.4, 1.0)
DOWNSAMPLE = 1

_PROG = None          # cached Bass program
LAST_RESULTS = None   # BassKernelResults of the most recent run (for test.py)


def _host_factors(pred_box_infra, infra_features):
    """Per-box scalars, bilinear-sampled box features and separable gaussian
    profiles - all tiny. Coordinate math in float32 to match the reference
    bit-for-bit where it matters (floor/clip decisions)."""
    boxes = pred_box_infra[:N].astype(np.float32)
    feat = infra_features[0]                      # [C,H,W] float32
    l_corner = boxes.min(axis=1)                  # [N,3]
    r_corner = boxes.max(axis=1)
    sx = np.float32(VOXEL[0] * DOWNSAMPLE)
    sy = np.float32(VOXEL[1] * DOWNSAMPLE)
    x1 = (l_corner[:, 0] - np.float32(LIDAR_RANGE[0])) / sx
    y1 = (l_corner[:, 1] - np.float32(LIDAR_RANGE[1])) / sy
    x2 = (r_corner[:, 0] - np.float32(LIDAR_RANGE[0])) / sx
    y2 = (r_corner[:, 1] - np.float32(LIDAR_RANGE[1])) / sy
    bev_size = (y2 - y1) * (x2 - x1)              # [N]
    cx = np.float32(0.5) * (x1 + x2)
    cy = np.float32(0.5) * (y1 + y2)

    # bilinear sample at (cy, cx), matching the reference's clip/floor
    y = np.clip(cy, 0.0, H - 1.0).astype(np.float32)
    x = np.clip(cx, 0.0, W - 1.0).astype(np.float32)
    yl = np.floor(y).astype(np.int32)
    xl = np.floor(x).astype(np.int32)
    yh = np.minimum(yl + 1, H - 1)
    xh = np.minimum(xl + 1, W - 1)
    ly = (y - yl).astype(np.float64)[None, :]     # [1,N]
    lx = (x - xl).astype(np.float64)[None, :]
    g = lambda yi, xi: feat[:, yi, xi].astype(np.float64)   # [C,N]
    box_feats = (g(yl, xl) * (1 - ly) * (1 - lx)
                 + g(yl, xh) * (1 - ly) * lx
                 + g(yh, xl) * ly * (1 - lx)
                 + g(yh, xh) * ly * lx)           # [C,N] float64

    denom = 2.0 * bev_size.astype(np.float64) ** 2          # [N]
    hh = np.arange(H, dtype=np.float64)
    ww = np.arange(W, dtype=np.float64)
    gy = np.exp(-((hh[None, :] - x1.astype(np.float64)[:, None]) ** 2) / denom[:, None])
    gx = np.exp(-(ww[None, :] ** 2) / denom[:, None])

    a_t = np.ascontiguousarray((box_feats / N).T.astype(np.float32))  # [N,C]
    return a_t, gy.astype(np.float32), gx.astype(np.float32)


def _build_program():
    nc = bass.Bass("TRN2", target_bir_lowering=False, debug=False,
                   num_devices=N_CORES)
    params = nc.dram_tensor("params", [N, PF], F16, kind="ExternalInput").ap()
    out = nc.dram_tensor("out", [C, HS, W], F16, kind="ExternalOutput").ap()
    # [c, b, h, w] view with c the 128-partition dim and b the c-half.
    out_v = out.rearrange("(b c) h w -> c b h w", b=2)

    with ExitStack() as ctx:
        tc = ctx.enter_context(tile.TileContext(nc))
        const = ctx.enter_context(tc.tile_pool(name="const", bufs=1))
        ppool = ctx.enter_context(tc.tile_pool(name="psum", bufs=4, space="PSUM"))
        # One stage pool per chunk size; bufs == #chunks of that size, so
        # stage slots are never recycled (no release waits needed at all).
        spools = {}
        for s in sorted(set(CHUNKS)):
            spools[s] = ctx.enter_context(
                tc.tile_pool(name=f"stage{s}", bufs=CHUNKS.count(s)))

        p_sb = const.tile([N, PF], F16)
        in_dma = nc.sync.dma_start(p_sb[:], params[:])
        b_all = p_sb[:, 0:HS * C]     # [N, h*256+c] premultiplied stationaries
        gx16 = p_sb[:, HS * C:PF]     # [N, W] fp16 gaussian row
        # One scratch column per chunk for the DVE joiner memsets.
        scratch = const.tile([128, len(CHUNKS)], F32)

        tail_deps = [in_dma.ins]
        h = 0
        for ci, s in enumerate(CHUNKS):
            # Stage layout per partition: [b(half)][h][w] so the DMA's SBUF
            # side merges (h,w) into one contiguous run and balances at 3D.
            stage = spools[s].tile([128, 2 * s * W], F16, tag="stage")
            stage_v = stage[:].rearrange("p (b h w) -> p b h w", b=2, h=s)
            for l in range(s):
                ps = ppool.tile([128, 2 * W], F32, tag="ps")
                nc.tensor.matmul(ps[:, 0:W],
                                 b_all[:, h * C:h * C + 128], gx16,
                                 start=True, stop=True)
                mm = nc.tensor.matmul(ps[:, W:2 * W],
                                      b_all[:, h * C + 128:(h + 1) * C], gx16,
                                      start=True, stop=True)
                ps_v = ps[:].rearrange("p (b w) -> p b w", b=2)
                if h % 2 == 0:
                    cp = nc.vector.tensor_copy(stage_v[:, :, l, :], ps_v)
                else:
                    cp = nc.scalar.copy(stage_v[:, :, l, :], ps_v)
                    last_act_cp = cp
                h += 1
            # The chunk DMA depends on copies from both engines, but the
            # DMA descriptor holds ONE sync wait. Emit a DVE joiner that
            # waits on the chunk's last Act copy (DVE program order already
            # covers the DVE copies); the DMA then waits only the joiner's
            # DVE tick, and the implied Act wait is elided post-assignment.
            joiner = nc.vector.memset(scratch[:, ci:ci + 1], 0.0)
            add_dep_helper(joiner.ins, last_act_cp.ins, sync=True,
                           reason="chunk copy joiner")
            dma = nc.sync.dma_start(out_v[:, :, h - s:h, :], stage_v)
            add_dep_helper(dma.ins, joiner.ins, sync=True,
                           reason="dma waits joiner")
            tail_deps.append(dma.ins)

        # Tail drain pre-cover: one single-wait SP nop per outstanding sem
        # so the drain itself needs no multi-wait instruction.
        tail_deps += [mm.ins, cp.ins, joiner.ins]
        for dep in tail_deps:
            tnop = nc.sync.nop(nofuse=True)
            add_dep_helper(tnop.ins, dep, sync=True,
                           reason="tail drain pre-cover")
    _elide_implied_waits(nc, tc)
    return nc


def _elide_implied_waits(nc, tc):
    """Several ISA structs (Matmult, TensorScalar, DMA_DIRECT2D) hold ONE
    sync wait, but Tile sometimes assigns two:
      - PSUM slot recycling puts both the PSUM->SBUF copy's tick and a WAW
        "previous writer retired" PE self-wait on the reusing matmul, yet
        the copy itself already waits for that PE tick;
      - a chunk DMA waits on both copy engines, yet its DVE joiner already
        waits the Act tick.
    A wait is provably redundant when another wait's producing instruction
    itself waits for the same semaphore at an equal-or-higher value
    (completion of the producer implies the dropped condition). Verify that
    per instruction and drop only implied waits."""
    # (sem_name, cumulative_value) -> instruction whose update reaches it.
    producer = {}
    cum = {}
    for insts in tc.ordered_instructions_by_block.values():
        for inst in insts:
            si = inst.sync_info
            if si is None:
                continue
            for u in si.on_update:
                cum[u.ant_name] = cum.get(u.ant_name, 0) + (u.update_value or 1)
                producer[(u.ant_name, cum[u.ant_name])] = inst

    def implied(keep, w):
        """True if wait `w` is implied by completion of `keep`'s producer."""
        prod = producer.get((keep.ant_name, keep.wait_value))
        if prod is None or prod.sync_info is None:
            return False
        return any(pw.ant_name == w.ant_name and pw.wait_value >= w.wait_value
                   for pw in prod.sync_info.on_wait)

    for inst in nc.inst_map.values():
        si = inst.sync_info
        if si is None or len(si.on_wait) < 2:
            continue
        waits = list(si.on_wait)
        changed = True
        while changed and len(waits) > 1:
            changed = False
            for w in waits:
                if any(k is not w and implied(k, w) for k in waits):
                    waits.remove(w)
                    changed = True
                    break
        if len(waits) != len(si.on_wait):
            si.on_wait = waits
            inst.sync_info = si


def _program():
    global _PROG
    if _PROG is None:
        _PROG = _build_program()
    return _PROG


def make_in_maps(pred_box_infra, infra_features):
    a_t, gy_full, gx = _host_factors(
        np.asarray(pred_box_infra, dtype=np.float32),
        np.asarray(infra_features, dtype=np.float32),
    )
    # B[n, h, c] = a_t[n, c] * gy[n, h], shipped premultiplied in fp16.
    b_full = (gy_full[:, :, None] * a_t[:, None, :]).astype(np.float16)
    gx16 = gx.astype(np.float16)
    return [
        {
            "params": np.ascontiguousarray(np.concatenate(
                [b_full[:, c * HS:(c + 1) * HS, :].reshape(N, HS * C), gx16],
                axis=1)),
        }
        for c in range(N_CORES)
    ]


def kernel(pred_box_infra, infra_features):
    global LAST_RESULTS
    in_maps = make_in_maps(pred_box_infra, infra_features)
    nc = _program()
    res = run_bass_kernel_spmd(nc, in_maps, core_ids=list(range(N_CORES)))
    LAST_RESULTS = res
    full = np.empty((1, C, H, W), dtype=np.float32)
    for c in range(N_CORES):
        full[0, :, c * HS:(c + 1) * HS, :] = res.results[c]["out"].astype(np.float32)
    return full


# revision 26
# speedup vs baseline: 3.7710x; 1.1112x over previous
"""Trainium2 Bass kernel for gaussian-weighted box-feature scatter (pooling).

Math (from the reference):
    out[c,h,w] = (1/N) * sum_n box_feats[c,n] * gmaps[n,h,w]
with gmaps separable:
    gmaps[n,h,w] = gy[n,h] * gx[n,w]

Host (tiny, O(N*C + N*(H+W) + N*H*C rank-factor prep)): box corner math, one
bilinear sample per box (box_feats [C,N]), the two 1-D gaussian profiles
gy [N,H], gx [N,W], and the premultiplied stationary factors
    B[n,h,c] = (box_feats[c,n]/N) * gy[n,h]   (fp16)
which ship to the device alongside gx (fp16) in one input DMA (~676 KB).

Device (heavy, O(C*H*W)): rank-N reconstruction
    out[c,h,:] = B[:,h,chalf].T @ gx
as 128 fp16 matmuls (stationary B slice via a ~105 ns standalone LDWEIGHTS
that the PE hides behind the previous matmul's streaming; moving gx), PSUM
f32 accumulate, fp16 PSUM->SBUF copy-casts, fp16 DMA writes. The f32->fp16
output is the big win: the kernel is write-bandwidth bound (per-core HBM
~358 GB/s), so halving output bytes halves the floor (16.8 MB/core ~ 47 us).
Host upcasts fp16 -> f32 while gathering.

Engine split (all under the ~47 us DMA window):
  PE:     128x (LDWEIGHTS + matmul [20,128]x[20,512] fp16)
  DVE:    even-h PSUM->SBUF double copies [128,1024] f32->fp16 (32x ~1.2 us)
  Scalar: odd-h double copies (32x ~1.1 us)
  SP:     1 input DMA + 7 chunked output DMAs (exactly 8 HWDGE lanes)

Each psum tile spans 2 banks: mm half0 -> cols 0:512, half1 -> 512:1024, so
one copy per h moves both c-halves. Output chunks [2,2,4,8,16,16,16] h-rows:
the first DMA issues ~2.5 us after the input lands; every chunk DMA covers
both halves via a strided dram AP. Per-chunk DVE "joiner" memsets plus a
post-assignment implied-wait elision keep every single-wait ISA struct
(Matmult, DMA descriptor) at one sync wait.

Sharding: H split across the 8 cores (64 rows each) - fully local.
"""

import numpy as np
from contextlib import ExitStack

from concourse import bass, tile, mybir
from concourse.tile import add_dep_helper
from concourse.bass_utils import run_bass_kernel_spmd

# Problem shapes (hardcoded per the task contract).
C, H, W = 256, 512, 512
N = 20
N_CORES = 8
HS = H // N_CORES          # 64 rows of the output per core
# Production is PE-paced (~150 GB/s), far under DMA bandwidth, so chunk
# sizes only matter at the edges: the first chunk just needs to issue early
# enough for the DMA to keep pace, and a tiny last chunk minimizes the
# un-overlapped tail drain. 5 output DMAs + 3 input DMAs = 8 HWDGE lanes.
CHUNKS = [16, 16, 16, 14, 2]        # h-rows per output DMA chunk
F32 = mybir.dt.float32
F16 = mybir.dt.float16
# Params live in three partition groups at bases 0/32/64 (the legal PE
# row-tile positions; quadrant 3 at 96 is unusable), each holding 22/22/20
# h-rows of premultiplied stationaries plus a copy of gx:
#   group g, partitions [32g, 32g+20): [B(nh x 256 c) | gx(512)] fp16.
# Three partition-sliced input DMAs load them concurrently.
HGRP = [(0, 0, 22), (32, 22, 22), (64, 44, 20)]   # (partition base, h0, nh)
GB = 22 * C                # B columns per group (max nh * C)
PF = GB + W

VOXEL = (0.4, 0.4, 4.0)
LIDAR_RANGE = (-102.4, -102.4, -3.0, 102.4, 102.4, 1.0)
DOWNSAMPLE = 1

_PROG = None          # cached Bass program
LAST_RESULTS = None   # BassKernelResults of the most recent run (for test.py)


def _host_factors(pred_box_infra, infra_features):
    """Per-box scalars, bilinear-sampled box features and separable gaussian
    profiles - all tiny. Coordinate math in float32 to match the reference
    bit-for-bit where it matters (floor/clip decisions)."""
    boxes = pred_box_infra[:N].astype(np.float32)
    feat = infra_features[0]                      # [C,H,W] float32
    l_corner = boxes.min(axis=1)                  # [N,3]
    r_corner = boxes.max(axis=1)
    sx = np.float32(VOXEL[0] * DOWNSAMPLE)
    sy = np.float32(VOXEL[1] * DOWNSAMPLE)
    x1 = (l_corner[:, 0] - np.float32(LIDAR_RANGE[0])) / sx
    y1 = (l_corner[:, 1] - np.float32(LIDAR_RANGE[1])) / sy
    x2 = (r_corner[:, 0] - np.float32(LIDAR_RANGE[0])) / sx
    y2 = (r_corner[:, 1] - np.float32(LIDAR_RANGE[1])) / sy
    bev_size = (y2 - y1) * (x2 - x1)              # [N]
    cx = np.float32(0.5) * (x1 + x2)
    cy = np.float32(0.5) * (y1 + y2)

    # bilinear sample at (cy, cx), matching the reference's clip/floor
    y = np.clip(cy, 0.0, H - 1.0).astype(np.float32)
    x = np.clip(cx, 0.0, W - 1.0).astype(np.float32)
    yl = np.floor(y).astype(np.int32)
    xl = np.floor(x).astype(np.int32)
    yh = np.minimum(yl + 1, H - 1)
    xh = np.minimum(xl + 1, W - 1)
    ly = (y - yl).astype(np.float64)[None, :]     # [1,N]
    lx = (x - xl).astype(np.float64)[None, :]
    g = lambda yi, xi: feat[:, yi, xi].astype(np.float64)   # [C,N]
    box_feats = (g(yl, xl) * (1 - ly) * (1 - lx)
                 + g(yl, xh) * (1 - ly) * lx
                 + g(yh, xl) * ly * (1 - lx)
                 + g(yh, xh) * ly * lx)           # [C,N] float64

    denom = 2.0 * bev_size.astype(np.float64) ** 2          # [N]
    hh = np.arange(H, dtype=np.float64)
    ww = np.arange(W, dtype=np.float64)
    gy = np.exp(-((hh[None, :] - x1.astype(np.float64)[:, None]) ** 2) / denom[:, None])
    gx = np.exp(-(ww[None, :] ** 2) / denom[:, None])

    a_t = np.ascontiguousarray((box_feats / N).T.astype(np.float32))  # [N,C]
    return a_t, gy.astype(np.float32), gx.astype(np.float32)


def _build_program():
    nc = bass.Bass("TRN2", target_bir_lowering=False, debug=False,
                   num_devices=N_CORES)
    params = nc.dram_tensor("params", [60, PF], F16, kind="ExternalInput").ap()
    out = nc.dram_tensor("out", [C, HS, W], F16, kind="ExternalOutput").ap()
    # [c, b, h, w] view with c the 128-partition dim and b the c-half.
    out_v = out.rearrange("(b c) h w -> c b h w", b=2)

    with ExitStack() as ctx:
        tc = ctx.enter_context(tile.TileContext(nc))
        const = ctx.enter_context(tc.tile_pool(name="const", bufs=1))
        ppool = ctx.enter_context(tc.tile_pool(name="psum", bufs=4, space="PSUM"))
        # One stage pool per chunk size; bufs == #chunks of that size, so
        # stage slots are never recycled (no release waits needed at all).
        spools = {}
        for s in sorted(set(CHUNKS)):
            spools[s] = ctx.enter_context(
                tc.tile_pool(name=f"stage{s}", bufs=CHUNKS.count(s)))

        p_sb = const.tile([128, PF], F16)
        in_dmas = [
            nc.sync.dma_start(p_sb[base:base + N, :], params[20 * g:20 * g + N, :])
            for g, (base, _, _) in enumerate(HGRP)
        ]
        # One scratch column per chunk for the DVE joiner memsets.
        scratch = const.tile([128, len(CHUNKS)], F32)

        tail_deps = [dma.ins for dma in in_dmas]
        h = 0
        for ci, s in enumerate(CHUNKS):
            # Stage layout per partition: [b(half)][h][w] so the DMA's SBUF
            # side merges (h,w) into one contiguous run and balances at 3D.
            stage = spools[s].tile([128, 2 * s * W], F16, tag="stage")
            stage_v = stage[:].rearrange("p (b h w) -> p b h w", b=2, h=s)
            for l in range(s):
                base, h0, _ = next(gr for gr in HGRP
                                   if gr[1] <= h < gr[1] + gr[2])
                bcol = (h - h0) * C
                bg = p_sb[base:base + N, :]
                ps = ppool.tile([128, 2 * W], F32, tag="ps")
                nc.tensor.matmul(ps[:, 0:W],
                                 bg[:, bcol:bcol + 128], bg[:, GB:PF],
                                 start=True, stop=True)
                mm = nc.tensor.matmul(ps[:, W:2 * W],
                                      bg[:, bcol + 128:bcol + C], bg[:, GB:PF],
                                      start=True, stop=True)
                ps_v = ps[:].rearrange("p (b w) -> p b w", b=2)
                if h % 2 == 0:
                    cp = nc.vector.tensor_copy(stage_v[:, :, l, :], ps_v)
                else:
                    cp = nc.scalar.copy(stage_v[:, :, l, :], ps_v)
                    last_act_cp = cp
                h += 1
            # The chunk DMA depends on copies from both engines, but the
            # DMA descriptor holds ONE sync wait. Emit a DVE joiner that
            # waits on the chunk's last Act copy (DVE program order already
            # covers the DVE copies); the DMA then waits only the joiner's
            # DVE tick, and the implied Act wait is elided post-assignment.
            joiner = nc.vector.memset(scratch[:, ci:ci + 1], 0.0)
            add_dep_helper(joiner.ins, last_act_cp.ins, sync=True,
                           reason="chunk copy joiner")
            dma = nc.sync.dma_start(out_v[:, :, h - s:h, :], stage_v)
            add_dep_helper(dma.ins, joiner.ins, sync=True,
                           reason="dma waits joiner")
            tail_deps.append(dma.ins)

        # Tail drain pre-cover: one single-wait SP nop per outstanding sem
        # so the drain itself needs no multi-wait instruction.
        tail_deps += [mm.ins, cp.ins, joiner.ins]
        for dep in tail_deps:
            tnop = nc.sync.nop(nofuse=True)
            add_dep_helper(tnop.ins, dep, sync=True,
                           reason="tail drain pre-cover")
    _elide_implied_waits(nc, tc)
    return nc


def _elide_implied_waits(nc, tc):
    """Several ISA structs (Matmult, TensorScalar, DMA_DIRECT2D) hold ONE
    sync wait, but Tile sometimes assigns two:
      - PSUM slot recycling puts both the PSUM->SBUF copy's tick and a WAW
        "previous writer retired" PE self-wait on the reusing matmul, yet
        the copy itself already waits for that PE tick;
      - a chunk DMA waits on both copy engines, yet its DVE joiner already
        waits the Act tick.
    A wait is provably redundant when another wait's producing instruction
    itself waits for the same semaphore at an equal-or-higher value
    (completion of the producer implies the dropped condition). Verify that
    per instruction and drop only implied waits."""
    # (sem_name, cumulative_value) -> instruction whose update reaches it.
    producer = {}
    cum = {}
    for insts in tc.ordered_instructions_by_block.values():
        for inst in insts:
            si = inst.sync_info
            if si is None:
                continue
            for u in si.on_update:
                cum[u.ant_name] = cum.get(u.ant_name, 0) + (u.update_value or 1)
                producer[(u.ant_name, cum[u.ant_name])] = inst

    def implied(keep, w):
        """True if wait `w` is implied by completion of `keep`'s producer."""
        prod = producer.get((keep.ant_name, keep.wait_value))
        if prod is None or prod.sync_info is None:
            return False
        return any(pw.ant_name == w.ant_name and pw.wait_value >= w.wait_value
                   for pw in prod.sync_info.on_wait)

    for inst in nc.inst_map.values():
        si = inst.sync_info
        if si is None or len(si.on_wait) < 2:
            continue
        waits = list(si.on_wait)
        changed = True
        while changed and len(waits) > 1:
            changed = False
            for w in waits:
                if any(k is not w and implied(k, w) for k in waits):
                    waits.remove(w)
                    changed = True
                    break
        if len(waits) != len(si.on_wait):
            si.on_wait = waits
            inst.sync_info = si


def _program():
    global _PROG
    if _PROG is None:
        _PROG = _build_program()
    return _PROG


def make_in_maps(pred_box_infra, infra_features):
    a_t, gy_full, gx = _host_factors(
        np.asarray(pred_box_infra, dtype=np.float32),
        np.asarray(infra_features, dtype=np.float32),
    )
    # B[n, h, c] = a_t[n, c] * gy[n, h], shipped premultiplied in fp16 in
    # three h-groups (one per PE row-tile position), each with a gx copy.
    b_full = (gy_full[:, :, None] * a_t[:, None, :]).astype(np.float16)
    gx16 = gx.astype(np.float16)
    maps = []
    for c in range(N_CORES):
        p = np.zeros((60, PF), dtype=np.float16)
        b_core = b_full[:, c * HS:(c + 1) * HS, :]       # [N, HS, C]
        for g, (_, h0, nh) in enumerate(HGRP):
            p[20 * g:20 * g + N, 0:nh * C] = \
                b_core[:, h0:h0 + nh, :].reshape(N, nh * C)
            p[20 * g:20 * g + N, GB:PF] = gx16
        maps.append({"params": p})
    return maps


def kernel(pred_box_infra, infra_features):
    global LAST_RESULTS
    in_maps = make_in_maps(pred_box_infra, infra_features)
    nc = _program()
    res = run_bass_kernel_spmd(nc, in_maps, core_ids=list(range(N_CORES)))
    LAST_RESULTS = res
    full = np.empty((1, C, H, W), dtype=np.float32)
    for c in range(N_CORES):
        full[0, :, c * HS:(c + 1) * HS, :] = res.results[c]["out"].astype(np.float32)
    return full
